# revision 48
# baseline (speedup 1.0000x reference)
"""CopyGenerator kernel for Trainium2, SPMD over 8 NeuronCores.

Problem (nn_CopyGenerator):
    logits = hidden @ W.T + b            # (N=4096, V=32000)
    prob   = softmax(logits, axis=1)
    p_copy = sigmoid(hidden @ Wc.T + bc) # (N, 1)
    out    = [prob * (1 - p_copy),  scatter(attn * p_copy)]   # (N, 32620)

Sharding: TENSOR-parallel over the vocab dim. Core k owns W columns
[4000k, 4000(k+1)) (resident in SBUF, ~4MB) and computes those logit
columns for ALL 4096 rows; the (hidden^T) activations are replicated.
This cuts per-core HBM traffic from ~136MB (batch-parallel, W streamed
twice) to ~78MB: the only large stream is the fp32 output itself.

The vocab bias b is NOT added on the PE (a K=1 bias matmul would cost a
full 500-cycle pass per psum tile, +25% PE time). Instead exp(l+b) =
exp(l)*exp(b): the host passes eb = exp(b); after the Exp activation a
single DVE scalar_tensor_tensor computes e*eb in place AND accumulates
the per-row partial softmax sums. Those partial sums are exchanged with
a small AllGather (cheaper than AllReduce in this fabric: no 1.875x
reduction factor) batched over "passes" of row-chunks; each core then
reduces the 8 gathered partials, takes the reciprocal, and scales its
vocab slice by (1-p_copy)*(1/Z) with one fused two-scalar DVE op into
fp32 for the store. The reduce+scale for pass p runs two passes later
(its AllGather is then guaranteed complete), so the in-order DVE queue
never head-of-line blocks on a collective; bounce-buffer DMAs ride the
gpsimd queue to stay clear of the store stream. p_copy terms use
Exp+reciprocal instead of Sigmoid so the activation table never swaps
mid-loop.

Row-chunk = 128 rows (one partition block); 32 chunks per core. Four
PSUM pair-tiles [128, 2, 512] per chunk hold 2x500 logit columns each
(each 500-wide matmul chain stays inside one 2KB PSUM bank); one Exp
activation drains a pair via a strided 2-group access pattern.
"""

import numpy as np

import concourse.mybir as mybir
import concourse.tile as tile
from concourse import bacc
from concourse.bass_utils import run_bass_kernel_spmd

# Problem constants (hardcoded per contract)
B, T, S, H, V, C = 32, 128, 400, 512, 32000, 620
N = B * T                  # 4096 rows
NCORES = 8
VS = V // NCORES           # vocab shard per core = 4000
BPC = B // NCORES          # batch elems per core = 4 (copy part only)
ROWS = BPC * T             # copy-part rows per core = 512
P = 128                    # partitions
NCH = N // P               # row chunks per core = 32
KK = H // P                # 4 contraction tiles
NG = VS // 1000            # 4 psum pair-tiles per chunk
NT = 500                   # columns per matmul chain (one PSUM bank)

# AllGather pass sizes (chunks per collective). Early passes start the
# store pipeline ASAP; the tail is shaped so the last AllGathers don't
# cascade on the collective engine past compute end.
PASS_SIZES = [4, 4, 4, 4, 4, 4, 4, 2, 2]
assert sum(PASS_SIZES) == NCH

FP16 = mybir.dt.float16
FP32 = mybir.dt.float32
AF = mybir.ActivationFunctionType
ALU = mybir.AluOpType

# scheduling knobs (swept in simulation)
CFG = {
    "zadd_eng": "vector",   # engine for the Z tree adds
    "pool_every": 0,        # 1/N of pass-B chunks on gpsimd (0 = none)
    "drip_reduce_at": 1,    # chunk index in next pass to emit the reduce
    "drip_b_from": 2,       # chunk index to start dripping pass-B work
    "act_every": 0,         # chunks with c%N in {1,3} scale on ACT (0 = none)
    "gen_lag": 2,           # generations between compute and reduce+scale
    "reduce_at": 99,         # chunk index for the reduce (99 = after stts)
    "split_tail": 99,       # split the pass-final stt only in the last N passes
    "split_n": 4,           # pieces for the pass-final stt (4 or 8)
    "bin_eng": "gpsimd",    # engine issuing the zpack upload DMA
    "zg_eng": "gpsimd",     # engine issuing the gathered-Z readback DMA
}


def build_kernel(bc_value: float):
    nc = bacc.Bacc("TRN2", target_bir_lowering=False, num_devices=NCORES)

    hT = nc.dram_tensor("hT", (H, N), FP16, kind="ExternalInput")
    wT = nc.dram_tensor("wT", (H, VS), FP16, kind="ExternalInput")
    ebr = nc.dram_tensor("ebr", (1, VS), FP16, kind="ExternalInput")
    wc4 = nc.dram_tensor("wc4", (P, KK), FP16, kind="ExternalInput")
    hTb = nc.dram_tensor("hTb", (H, ROWS), FP16, kind="ExternalInput")
    aT = nc.dram_tensor("aT", (BPC, S, T), FP16, kind="ExternalInput")
    sm = nc.dram_tensor("sm", (BPC, S, C), FP16, kind="ExternalInput")
    vout = nc.dram_tensor("vout", (N, VS), FP32, kind="ExternalOutput")
    cout = nc.dram_tensor("cout", (ROWS, C), FP32, kind="ExternalOutput")

    with tile.TileContext(nc) as tc:
        with (
            tc.tile_pool(name="const", bufs=1) as const,
            tc.tile_pool(name="epool", bufs=10) as epool,
            tc.tile_pool(name="stp", bufs=2) as stp,
            tc.tile_pool(name="smallp", bufs=4) as smallp,
            tc.tile_pool(name="zpool", bufs=2) as zpool,
            tc.tile_pool(name="cpin", bufs=1) as cpin,
            tc.tile_pool(name="cstp", bufs=1) as cstp,
            tc.tile_pool(name="psmain", bufs=3, space="PSUM") as psmain,
            tc.tile_pool(name="psaux", bufs=1, space="PSUM") as psaux,
            tc.tile_pool(name="dram", bufs=2, space="DRAM") as dram,
        ):
            # ---- resident loads, ordered for earliest PE start ----
            wc_sb = const.tile([P, KK], FP16, name="wc_sb")
            nc.sync.dma_start(wc_sb[:], wc4[:, :])
            ebr_sb = const.tile([1, VS], FP16, name="ebr_sb")
            nc.sync.dma_start(ebr_sb[:], ebr[0:1, :])
            hts = [const.tile([P, N], FP16, name=f"ht{kk}") for kk in range(KK)]
            wts = [const.tile([P, VS], FP16, name=f"wt{kk}") for kk in range(KK)]
            hbs = [const.tile([P, ROWS], FP16, name=f"hb{kk}")
                   for kk in range(KK)]
            # chunk 0's needs first: W quarter 0 + hidden eighth 0, then the
            # rest of W, then hTb (copy-gate rows), then the hidden remainder
            E8 = N // 8
            Q4 = VS // 4
            for kk in range(KK):
                nc.sync.dma_start(wts[kk][:, 0:Q4], wT[kk * P:(kk + 1) * P, 0:Q4])
            for kk in range(KK):
                nc.sync.dma_start(hts[kk][:, 0:E8], hT[kk * P:(kk + 1) * P, 0:E8])
            for q in range(1, 4):
                for kk in range(KK):
                    nc.sync.dma_start(
                        wts[kk][:, q * Q4:(q + 1) * Q4],
                        wT[kk * P:(kk + 1) * P, q * Q4:(q + 1) * Q4],
                    )
            for kk in range(KK):
                nc.sync.dma_start(hbs[kk][:], hTb[kk * P:(kk + 1) * P, :])
            for e8 in range(1, 8):
                for kk in range(KK):
                    nc.sync.dma_start(
                        hts[kk][:, e8 * E8:(e8 + 1) * E8],
                        hT[kk * P:(kk + 1) * P, e8 * E8:(e8 + 1) * E8],
                    )

            ones = const.tile([1, P], FP16, name="ones")
            nc.vector.memset(ones[:], 1.0)
            bc_pos = const.tile([P, 1], FP32, name="bc_pos")
            nc.vector.memset(bc_pos[:], bc_value)
            bc_neg = const.tile([P, 1], FP32, name="bc_neg")
            nc.vector.memset(bc_neg[:], -bc_value)
            ebt = const.tile([P, VS], FP16, name="ebt")
            ompcp = const.tile([P, NCH], FP32, name="ompcp")
            pcb = const.tile([P, BPC], FP32, name="pcb")

            # ---- eb broadcast to all partitions (PE) ----
            for g in range(NG):
                pa = psaux.tile([P, 2, 512], FP32, name="pa", tag="pa")
                for s_ in range(2):
                    cs = slice(g * 1000 + s_ * NT, g * 1000 + (s_ + 1) * NT)
                    nc.tensor.matmul(
                        pa[:, s_, 0:NT], ones[:], ebr_sb[0:1, cs],
                        start=True, stop=True,
                    )
                nc.scalar.activation(
                    ebt[:, g * 1000:(g + 1) * 1000], pa[:, :, 0:NT], AF.Copy
                )

            def emit_pcb():
                """p_copy for the copy part (batch-ordered rows):
                pc = sigmoid(y) = 1/(1+exp(-y)); avoids the Sigmoid table.
                Deferred past pass 0 so its hTb wait never heads the PE
                queue during warm-up."""
                pa = psaux.tile([P, 2, 512], FP32, name="pa", tag="pa")
                for j in range(BPC):
                    js = slice(j * P, (j + 1) * P)
                    for kk in range(KK):
                        nc.tensor.matmul(
                            pa[:, 0, j:j + 1], hbs[kk][:, js],
                            wc_sb[:, kk:kk + 1],
                            start=(kk == 0), stop=(kk == KK - 1),
                        )
                ub = smallp.tile([P, BPC], FP32, name="ub", tag="ub")
                nc.scalar.activation(ub[:], pa[:, 0, 0:BPC], AF.Exp,
                                     bias=bc_neg[:], scale=-1.0)
                ub1 = smallp.tile([P, BPC], FP32, name="ub1", tag="ub1")
                nc.vector.tensor_scalar_add(ub1[:], ub[:], 1.0)
                nc.vector.reciprocal(pcb[:], ub1[:])

            nks = (S + P - 1) // P

            def emit_copy(j):
                """copy/scatter output: p_copy * (attn_j @ src_map_j).
                aT/sm tiles are loaded just-in-time to keep SBUF small."""
                js = slice(j * P, (j + 1) * P)
                cp = psaux.tile([P, 2, 512], FP32, name="pa", tag="pa")
                for kkc in range(nks):
                    pk = min(P, S - kkc * P)
                    at = cpin.tile([P, T], FP16, name="at", tag=f"at{kkc}")
                    nc.sync.dma_start(at[:pk, :], aT[j, kkc * P:kkc * P + pk, :])
                    smt = cpin.tile([P, C], FP16, name="smt", tag=f"sm{kkc}")
                    nc.sync.dma_start(smt[:pk, :], sm[j, kkc * P:kkc * P + pk, :])
                    nc.tensor.matmul(
                        cp[:, 0, 0:512], at[:pk, :], smt[:pk, 0:512],
                        start=(kkc == 0), stop=(kkc == nks - 1),
                    )
                    nc.tensor.matmul(
                        cp[:, 1, 0:C - 512], at[:pk, :], smt[:pk, 512:C],
                        start=(kkc == 0), stop=(kkc == nks - 1),
                    )
                cst = cstp.tile([P, C], FP32, name="cst", tag="cst")
                nc.vector.tensor_scalar_mul(cst[:, 0:512], cp[:, 0, 0:512],
                                            pcb[:, j:j + 1])
                nc.vector.tensor_scalar_mul(cst[:, 512:C], cp[:, 1, 0:C - 512],
                                            pcb[:, j:j + 1])
                nc.sync.dma_start(cout[js, :], cst[:])

            # ---- main loop state ----
            e_tiles = [None] * NCH          # fp16 (128, VS): exp, then exp*eb
            eng_ns = {"dve": 0.0, "act": 0.0}

            def emit_pass_pc(chunks):
                """(1 - p_copy) for a pass's chunks in one PSUM tile:
                ompc = 1/(1+exp(y+bc)), batched m wide."""
                m = len(chunks)
                c0 = chunks[0]
                pcps = psaux.tile([P, 2, 512], FP32, name="pa", tag="pa")
                for i, c in enumerate(chunks):
                    rs = slice(c * P, (c + 1) * P)
                    for kk in range(KK):
                        nc.tensor.matmul(
                            pcps[:, 0, i:i + 1], hts[kk][:, rs], wc_sb[:, kk:kk + 1],
                            start=(kk == 0), stop=(kk == KK - 1),
                        )
                uc = smallp.tile([P, 8], FP32, name="uc", tag="uc")
                nc.scalar.activation(uc[:, 0:m], pcps[:, 0, 0:m], AF.Exp,
                                     bias=bc_pos[:], scale=1.0)
                uc1 = smallp.tile([P, 8], FP32, name="uc1", tag="uc1")
                nc.vector.tensor_scalar_add(uc1[:, 0:m], uc[:, 0:m], 1.0)
                nc.vector.reciprocal(ompcp[:, c0:c0 + m], uc1[:, 0:m])

            def emit_chunk(c, zpack, zi, split_stt):
                """Matmul+exp chunk c; stt folds eb in-place and fills
                zpack[:, zi] with the weighted row-sum partial."""
                rs = slice(c * P, (c + 1) * P)
                e = epool.tile([P, VS], FP16, name="e", tag="e")
                e_tiles[c] = e
                for g in range(NG):
                    pp = psmain.tile([P, 2, 512], FP32, name="pp", tag="pp")
                    for s_ in range(2):
                        cs = slice(g * 1000 + s_ * NT, g * 1000 + (s_ + 1) * NT)
                        for kk in range(KK):
                            nc.tensor.matmul(
                                pp[:, s_, 0:NT], hts[kk][:, rs], wts[kk][:, cs],
                                start=(kk == 0), stop=(kk == KK - 1),
                            )
                    nc.scalar.activation(
                        e[:, g * 1000:(g + 1) * 1000], pp[:, :, 0:NT], AF.Exp
                    )
                if split_stt:
                    # last chunk of a pass: per-piece stt so the AllGather
                    # gate opens as soon as the last exp lands
                    np_ = CFG["split_n"]
                    w_ = VS // np_
                    zparts = smallp.tile([P, 8], FP32, name="zparts", tag="zparts")
                    for g in range(np_):
                        gs = slice(g * w_, (g + 1) * w_)
                        nc.vector.scalar_tensor_tensor(
                            e[:, gs], e[:, gs], 1.0, ebt[:, gs],
                            ALU.mult, ALU.mult, accum_out=zparts[:, g:g + 1],
                        )
                    if np_ == 8:
                        zq = smallp.tile([P, 4], FP32, name="zq", tag="zq")
                        nc.vector.tensor_add(zq[:], zparts[:, 0:4], zparts[:, 4:8])
                    else:
                        zq = zparts
                    zh = smallp.tile([P, 2], FP32, name="zh", tag="zh")
                    nc.vector.tensor_add(zh[:], zq[:, 0:2], zq[:, 2:4])
                    nc.vector.tensor_add(zpack[:, zi:zi + 1], zh[:, 0:1], zh[:, 1:2])
                    eng_ns["dve"] += np_ * 1170 + 160
                else:
                    nc.vector.scalar_tensor_tensor(
                        e[:], e[:], 1.0, ebt[:], ALU.mult, ALU.mult,
                        accum_out=zpack[:, zi:zi + 1],
                    )
                    eng_ns["dve"] += 4227

            pending_b = None

            def emit_reduce(chunks, zg, flags):
                """Reduce the 8 gathered partials. The tree adds run on
                gpsimd (its queue just finished the zg readback anyway);
                only a tiny reciprocal touches the DVE queue, so DVE never
                head-of-line blocks on the collective."""
                nonlocal pending_b
                m = len(chunks)
                zeng = getattr(nc, CFG["zadd_eng"])
                t1 = smallp.tile([P, 4 * m], FP32, name="t1", tag=f"t1{m}")
                zeng.tensor_add(t1[:], zg[:, 0:4 * m], zg[:, 4 * m:8 * m])
                t2 = smallp.tile([P, 2 * m], FP32, name="t2", tag=f"t2{m}")
                zeng.tensor_add(t2[:], t1[:, 0:2 * m], t1[:, 2 * m:4 * m])
                zs = smallp.tile([P, m], FP32, name="zs", tag=f"zs{m}")
                zeng.tensor_add(zs[:], t2[:, 0:m], t2[:, m:2 * m])
                rz = zpool.tile([P, m], FP32, name="rz", tag=f"rz{m}")
                nc.vector.reciprocal(rz[:], zs[:])
                pending_b = (chunks, rz, flags)

            def emit_b_chunk(c, ompc_c, rz_c, eng, s_c=None):
                """st = (e2 * (1-p_copy)) * (1/Z), fused two-scalar op,
                then one contiguous 2MB store."""
                st = stp.tile([P, VS], FP32, name="st", tag="st")
                e = e_tiles[c]
                if eng == "act":
                    H2 = VS // 2
                    nc.scalar.activation(st[:, 0:H2], e[:, 0:H2], AF.Copy,
                                         scale=s_c)
                    nc.scalar.activation(st[:, H2:VS], e[:, H2:VS], AF.Copy,
                                         scale=s_c)
                elif eng == "pool":
                    nc.gpsimd.tensor_scalar(st[:], e[:], ompc_c, rz_c,
                                            ALU.mult, ALU.mult)
                else:
                    nc.vector.tensor_scalar(st[:], e[:], ompc_c, rz_c,
                                            ALU.mult, ALU.mult)
                e_tiles[c] = None
                nc.sync.dma_start(vout[c * P:(c + 1) * P, :], st[:])

            def emit_gen_reduce(gB):
                """zg readback + Z tree-add + reciprocal + s for one
                generation (its AllGather must be complete or nearly so)."""
                mm = len(gB["chunks"])
                zg = smallp.tile([P, NCORES * mm], FP32,
                                 name="zg", tag=f"zg{mm}")
                getattr(nc, CFG["zg_eng"]).dma_start(
                    zg[:], gB["bout"][:, :, :].transpose([1, 0, 2]))
                t1 = smallp.tile([P, 4 * mm], FP32, name="t1", tag=f"t1{mm}")
                nc.vector.tensor_add(t1[:], zg[:, 0:4 * mm],
                                     zg[:, 4 * mm:8 * mm])
                t2 = smallp.tile([P, 2 * mm], FP32, name="t2", tag=f"t2{mm}")
                nc.vector.tensor_add(t2[:], t1[:, 0:2 * mm],
                                     t1[:, 2 * mm:4 * mm])
                zs = smallp.tile([P, mm], FP32, name="zs", tag=f"zs{mm}")
                nc.vector.tensor_add(zs[:], t2[:, 0:mm], t2[:, mm:2 * mm])
                rz = zpool.tile([P, mm], FP32, name="rz", tag=f"rz{mm}")
                nc.vector.reciprocal(rz[:], zs[:])
                gc0 = gB["chunks"][0]
                s_p = zpool.tile([P, mm], FP32, name="sp", tag=f"sp{mm}")
                nc.vector.tensor_mul(s_p[:], rz[:], ompcp[:, gc0:gc0 + mm])
                gB["rz"] = rz
                gB["s"] = s_p

            # ---- main pipeline over passes ----
            # Generation pipeline per pass p (passes are 3-4 chunks):
            #   pass p:   compute chunks, stt partial sums -> zpack(p),
            #             bin(p) upload [SP queue], AllGather(p) dispatch
            #             [pool queue; its SEQ is freed during the CC run]
            #   pass p+1: pool-side scale work of pass p-1 (data ready, so
            #             it never delays an AllGather), zg(p) readback
            #             [pool queue, waits AG(p) without blocking later
            #             AGs], Z tree-add + reciprocal [DVE, dripped after
            #             2 chunks so the in-order DVE queue never stalls],
            #             DVE-side scale+store of pass p dripped between
            #             remaining chunks.
            npass = len(PASS_SIZES)
            gens = []  # per pass: dict(chunks, bout, zg, rz, flags)
            c0 = 0
            for pi, m in enumerate(PASS_SIZES):
                chunks = list(range(c0, c0 + m))
                c0 += m
                emit_pass_pc(chunks)
                zpack = zpool.tile([P, m], FP32, name="zpack", tag=f"zp{m}")
                dve_bq = []
                lag = CFG["gen_lag"]
                gB = gens[-lag] if len(gens) >= lag else None
                for i, c in enumerate(chunks):
                    emit_chunk(c, zpack, i,
                               split_stt=(i == m - 1
                                          and pi >= npass - CFG["split_tail"]))
                    if i == CFG["reduce_at"] and gB is not None:
                        emit_gen_reduce(gB)
                        dve_bq = [(pc_, k) for k, pc_ in enumerate(gB["chunks"])]
                    elif i >= CFG["drip_b_from"] and dve_bq:
                        pc_, k = dve_bq.pop(0)
                        emit_b_chunk(pc_, ompcp[:, pc_:pc_ + 1],
                                     gB["rz"][:, k:k + 1], gB["flags"][k],
                                     gB["s"][:, k:k + 1])
                if gB is not None and gB["rz"] is None:
                    emit_gen_reduce(gB)
                    dve_bq = [(pc_, k) for k, pc_ in enumerate(gB["chunks"])]
                for pc_, k in dve_bq:
                    emit_b_chunk(pc_, ompcp[:, pc_:pc_ + 1],
                                 gB["rz"][:, k:k + 1], gB["flags"][k],
                                 gB["s"][:, k:k + 1])
                if pi == 1:
                    emit_pcb()
                if 1 <= pi <= BPC:
                    emit_copy(pi - 1)
                # collective for this pass
                bin_ = dram.tile([P, m], FP32, name="bin", tag=f"bin{m}")
                bout = dram.tile([NCORES, P, m], FP32, name="bout", tag=f"bout{m}")
                getattr(nc, CFG["bin_eng"]).dma_start(bin_[:], zpack[:])
                nc.gpsimd.collective_compute(
                    "AllGather", ALU.bypass,
                    replica_groups=[list(range(NCORES))],
                    ins=[bin_[:].opt()], outs=[bout[:].opt()],
                )
                ae_ = CFG["act_every"]
                pe_ = CFG["pool_every"]
                flags = []
                for k, cc_ in enumerate(chunks):
                    if ae_ and (cc_ % ae_) in (1, 3):
                        flags.append("act")
                    elif pe_ and pi < npass - 2 and (k % pe_ == 1):
                        flags.append("pool")
                    else:
                        flags.append("dve")
                gens.append({"chunks": chunks, "bout": bout, "rz": None,
                             "s": None, "flags": flags})

            # ---- epilogue: drain remaining generations ----
            for gi in range(-CFG["gen_lag"], 0):
                g = gens[gi]
                mm = len(g["chunks"])
                if g["rz"] is None:
                    zg = smallp.tile([P, NCORES * mm], FP32,
                                     name="zg", tag=f"zge{mm}")
                    getattr(nc, CFG["zg_eng"]).dma_start(
                        zg[:], g["bout"][:, :, :].transpose([1, 0, 2]))
                    t1 = smallp.tile([P, 4 * mm], FP32, name="t1", tag=f"t1e{mm}")
                    nc.vector.tensor_add(t1[:], zg[:, 0:4 * mm],
                                         zg[:, 4 * mm:8 * mm])
                    t2 = smallp.tile([P, 2 * mm], FP32, name="t2", tag=f"t2e{mm}")
                    nc.vector.tensor_add(t2[:], t1[:, 0:2 * mm],
                                         t1[:, 2 * mm:4 * mm])
                    zs = smallp.tile([P, mm], FP32, name="zs", tag=f"zse{mm}")
                    nc.vector.tensor_add(zs[:], t2[:, 0:mm], t2[:, mm:2 * mm])
                    rz = zpool.tile([P, mm], FP32, name="rz", tag=f"rze{mm}")
                    nc.vector.reciprocal(rz[:], zs[:])
                    gc0 = g["chunks"][0]
                    s_p = zpool.tile([P, mm], FP32, name="sp", tag=f"spe{mm}")
                    nc.vector.tensor_mul(s_p[:], rz[:], ompcp[:, gc0:gc0 + mm])
                    g["rz"] = rz
                    g["s"] = s_p
                for k, pc_ in enumerate(g["chunks"]):
                    if e_tiles[pc_] is not None:
                        emit_b_chunk(pc_, ompcp[:, pc_:pc_ + 1],
                                     g["rz"][:, k:k + 1], g["flags"][k],
                                     g["s"][:, k:k + 1])

    nc.finalize()
    return nc


def _prep_inputs(hidden, attn, W, b, Wc, bc, src_map):
    """Host-side shard + layout prep. Returns per-core input maps and bc."""
    hidden, attn, W, b, Wc, bc, src_map = (
        np.asarray(x) for x in (hidden, attn, W, b, Wc, bc, src_map)
    )
    f16 = np.float16
    hT = np.ascontiguousarray(hidden.T.astype(f16))              # (512, 4096)
    eb = np.exp(b.astype(np.float64)).astype(f16)                # (32000,)
    wc4 = np.ascontiguousarray(Wc[0].reshape(KK, P).T.astype(f16))
    hid = hidden.reshape(T, B, H)
    att = attn.reshape(T, B, S)

    in_maps = []
    for k in range(NCORES):
        cs = slice(k * VS, (k + 1) * VS)
        bs = slice(k * BPC, (k + 1) * BPC)
        wT_k = np.ascontiguousarray(W[cs].T.astype(f16))         # (512, 4000)
        ebr_k = np.ascontiguousarray(eb[cs].reshape(1, VS))
        hb = hid[:, bs, :].transpose(1, 0, 2).reshape(ROWS, H)   # (512, 512)
        hTb_k = np.ascontiguousarray(hb.T.astype(f16))
        aT_k = np.ascontiguousarray(
            att[:, bs, :].transpose(1, 2, 0).astype(f16))        # (4, S, T)
        sm_k = np.ascontiguousarray(
            src_map[:, bs, :].transpose(1, 0, 2).astype(f16))    # (4, S, C)
        in_maps.append({"hT": hT, "wT": wT_k, "ebr": ebr_k, "wc4": wc4,
                        "hTb": hTb_k, "aT": aT_k, "sm": sm_k})
    return in_maps, float(bc[0])


def _assemble(results):
    """Per-core (4096, 4000) vocab slices + (512, 620) copy slices ->
    full (4096, 32620)."""
    out = np.empty((N, V + C), dtype=np.float32)
    for k, r in enumerate(results):
        out[:, k * VS:(k + 1) * VS] = r["vout"]
    ocv = out[:, V:].reshape(T, B, C)
    for k, r in enumerate(results):
        ocv[:, k * BPC:(k + 1) * BPC, :] = (
            r["cout"].reshape(BPC, T, C).transpose(1, 0, 2))
    return out


_CACHE = {}


def _run(inputs, **spmd_kwargs):
    in_maps, bc_value = _prep_inputs(**inputs)
    key = round(bc_value, 12)
    if key not in _CACHE:
        _CACHE[key] = build_kernel(bc_value)
    nc = _CACHE[key]
    res = run_bass_kernel_spmd(
        nc, in_maps, core_ids=list(range(NCORES)), **spmd_kwargs
    )
    return _assemble(res.results), res


def kernel(**inputs):
    out, _ = _run(inputs)
    return out
